# revision 6
# baseline (speedup 1.0000x reference)
# Trainium2 Bass kernel for nn_AxonalConnections (gnn_message_passing).
#
# Computes out[B, H, W] = (spikes.reshape(B, N) @ adjacency.T).reshape(B, H, W)
# with B=16, H=W=128, N=16384 on 8 NeuronCores.
#
# Strategy (pure tensor parallelism, no collectives):
#   - Shard adjacency row-wise (target dim) across 8 cores: core i owns
#     target columns [i*2048, (i+1)*2048) of the output.
#   - Host-side, transpose each shard to [source, target] layout so the
#     contraction dim (source) lands on SBUF partitions with unit-stride DMAs.
#   - The kernel is HBM-bandwidth bound, so minimize bytes: adjacency is
#     shipped as fp16 (values are ~N(0, 0.02^2), well inside fp16 range;
#     2^-11 relative representation error -> ~1e-4 output error). fp32
#     matmul would also stream 4x slower through the PE; fp16 streams at
#     full rate (1 column/cycle).
#   - Spikes (tiny) are split into fp16 hi + fp16 lo (exact to ~2^-22) and
#     packed as the stationary operand [spikes_hi | spikes_lo] (32 columns).
#     PSUM accumulates [32, 2048] fp32; rows 0-15 = hi terms, rows 16-31 =
#     lo terms; host folds them and concatenates the target shards.
#
# Per-core traffic: 64 MiB adjacency + 1 MiB spikes; single-queue HWDGE DMA
# sustains ~420 GB/s -> ~155 us steady state + ~25 us head/tail.

import numpy as np

B = 16
H = 128
W = 128
N = H * W            # 16384 source == target size
NCORES = 8
TSH = N // NCORES    # 2048 target columns per core
P = 128              # SBUF partitions / contraction tile
SCHUNKS = N // P     # 128 source chunks
GROUP = 4            # source chunks per DMA (GROUP * 0.5 MiB per transfer)
NFREE = 512          # matmul moving free dim (one PSUM bank of fp32)

_cache = {}


def _build_nc():
    import concourse.mybir as mybir
    import concourse.tile as tile
    from concourse import bacc

    nc = bacc.Bacc(
        "TRN2",
        target_bir_lowering=False,
        debug=False,
        num_devices=NCORES,
    )
    # a16: adjacency shard, transposed to [source, target] fp16, with two
    # source-chunks packed per DRAM row so every DMA descriptor moves a
    # contiguous 8 KiB run per partition (4 KiB runs halve DMA throughput:
    # the SDMA per-packet overhead is ~12 ns regardless of size).
    #   a16[g2*128 + p, half*TSH + t] = fp16(adj[t0 + t, (2*g2 + half)*128 + p])
    a16 = nc.dram_tensor(
        "a16", [N // 2, 2 * TSH], mybir.dt.float16, kind="ExternalInput"
    ).ap()
    # spk: stationary weights, packed [P, SCHUNKS*32] fp16 where
    #   spk[p, n*32 + b]      = fp16_hi(spikes[b, n*128 + p])
    #   spk[p, n*32 + 16 + b] = fp16_lo(spikes[b, n*128 + p])
    spk = nc.dram_tensor(
        "spk", [P, SCHUNKS * 32], mybir.dt.float16, kind="ExternalInput"
    ).ap()
    out = nc.dram_tensor("o", [32, TSH], mybir.dt.float32, kind="ExternalOutput").ap()

    f32 = mybir.dt.float32
    f16 = mybir.dt.float16
    NJ = TSH // NFREE  # 4 PSUM banks

    with tile.TileContext(nc) as tc:
        with (
            tc.tile_pool(name="adj", bufs=5) as adj_pool,
            tc.tile_pool(name="spkp", bufs=1) as spk_pool,
            tc.tile_pool(name="psum", bufs=1, space="PSUM") as psum_pool,
            tc.tile_pool(name="outp", bufs=1) as out_pool,
        ):
            # Load the stationary weights via the gpsimd (SWDGE) path so the
            # transfer overlaps with the first adjacency DMAs on the HWDGE
            # queue instead of serializing ahead of them.
            spk_t = spk_pool.tile([P, SCHUNKS * 32], f16)
            nc.gpsimd.dma_start(spk_t[:], spk[:])

            ps = psum_pool.tile([32, TSH], f32)

            ngroups = SCHUNKS // GROUP
            rows = (GROUP // 2) * P  # packed DRAM rows per group
            for g in range(ngroups):
                at = adj_pool.tile([P, GROUP * TSH], f16)
                # Alternate the two HWDGE rings (SP / ACT) so descriptor
                # generation for one group overlaps the other's transfer.
                dma_eng = nc.sync if g % 2 == 0 else nc.scalar
                dma_eng.dma_start(
                    at[:].rearrange("p (n t) -> p n t", n=GROUP // 2),
                    a16[g * rows : (g + 1) * rows, :].rearrange(
                        "(n p) t -> p n t", p=P
                    ),
                )
                for nl in range(GROUP):
                    n = g * GROUP + nl
                    w = spk_t[:, n * 32 : (n + 1) * 32]
                    base = nl * TSH
                    for j in range(NJ):
                        c0 = base + j * NFREE
                        nc.tensor.matmul(
                            ps[:, j * NFREE : (j + 1) * NFREE],
                            w,
                            at[:, c0 : c0 + NFREE],
                            start=(n == 0),
                            stop=(n == SCHUNKS - 1),
                        )

            ot = out_pool.tile([32, TSH], f32)
            nc.vector.tensor_copy(ot[:], ps[:])
            nc.sync.dma_start(out[:], ot[:])

    nc.compile()
    return nc


def _split_hi_lo(x32):
    """Split fp32 array into (hi, lo) fp16 parts with x32 ~= hi + lo."""
    hi = x32.astype(np.float16)
    lo = (x32 - hi.astype(np.float32)).astype(np.float16)
    return hi, lo


def _prep_inputs(spikes, adjacency):
    flat = np.ascontiguousarray(np.asarray(spikes, dtype=np.float32).reshape(B, N))
    adj = np.asarray(adjacency, dtype=np.float32)

    flatT = np.ascontiguousarray(flat.T)  # [N, B]
    fhi, flo = _split_hi_lo(flatT)
    spk = np.empty((SCHUNKS, P, 32), np.float16)  # [n, p, 2*B]
    spk[:, :, :B] = fhi.reshape(SCHUNKS, P, B)
    spk[:, :, B:] = flo.reshape(SCHUNKS, P, B)
    spk = np.ascontiguousarray(spk.transpose(1, 0, 2)).reshape(P, SCHUNKS * 32)

    adjT = adj.T  # [source, target] view (strided)
    in_maps = []
    for i in range(NCORES):
        a16 = adjT[:, i * TSH : (i + 1) * TSH].astype(np.float16)  # [N, TSH]
        # Pack two source-chunks per DRAM row (see kernel comment).
        a16 = np.ascontiguousarray(
            a16.reshape(N // (2 * P), 2, P, TSH).transpose(0, 2, 1, 3)
        ).reshape(N // 2, 2 * TSH)
        in_maps.append({"a16": a16, "spk": spk})
    return in_maps


def _run(in_maps, **kwargs):
    from concourse.bass_utils import run_bass_kernel_spmd

    if "nc" not in _cache:
        _cache["nc"] = _build_nc()
    return run_bass_kernel_spmd(
        _cache["nc"], in_maps, core_ids=list(range(NCORES)), **kwargs
    )


def kernel(spikes, adjacency):
    in_maps = _prep_inputs(spikes, adjacency)
    res = _run(in_maps)
    outs = [r["o"] for r in res.results]
    # Fold hi-weight rows (0:16) + lo-weight rows (16:32), concat target shards.
    full = np.concatenate([o[:B] + o[B:] for o in outs], axis=1)  # [B, N]
    return np.ascontiguousarray(full.reshape(B, H, W), dtype=np.float32)


# revision 8
# speedup vs baseline: 1.1725x; 1.1725x over previous
# Trainium2 Bass kernel for nn_AxonalConnections (gnn_message_passing).
#
# Computes out[B, H, W] = (spikes.reshape(B, N) @ adjacency.T).reshape(B, H, W)
# with B=16, H=W=128, N=16384 on 8 NeuronCores.
#
# Strategy (pure tensor parallelism, no collectives):
#   - Shard adjacency row-wise (target dim) across 8 cores: core i owns
#     target columns [i*2048, (i+1)*2048) of the output.
#   - Host-side, transpose each shard to [source, target] layout so the
#     contraction dim (source) lands on SBUF partitions with unit-stride DMAs.
#   - The kernel is HBM-bandwidth bound, so minimize bytes: adjacency is
#     shipped as fp16 (values are ~N(0, 0.02^2), well inside fp16 range;
#     2^-11 relative representation error -> ~1e-4 output error). fp32
#     matmul would also stream 4x slower through the PE; fp16 streams at
#     full rate (1 column/cycle).
#   - Spikes (tiny) are split into fp16 hi + fp16 lo (exact to ~2^-22) and
#     packed as the stationary operand [spikes_hi | spikes_lo] (32 columns).
#     PSUM accumulates [32, 2048] fp32; rows 0-15 = hi terms, rows 16-31 =
#     lo terms; host folds them and concatenates the target shards.
#
# Per-core traffic: 64 MiB adjacency + 1 MiB spikes; single-queue HWDGE DMA
# sustains ~420 GB/s -> ~155 us steady state + ~25 us head/tail.

import numpy as np

B = 16
H = 128
W = 128
N = H * W            # 16384 source == target size
NCORES = 8
TSH = N // NCORES    # 2048 target columns per core
P = 128              # SBUF partitions / contraction tile
SCHUNKS = N // P     # 128 source chunks
GROUP = 4            # source chunks per DMA (GROUP * 0.5 MiB per transfer)
NFREE = 512          # matmul moving free dim (one PSUM bank of fp32)

_cache = {}


def _build_nc():
    import concourse.mybir as mybir
    import concourse.tile as tile
    from concourse import bacc

    nc = bacc.Bacc(
        "TRN2",
        target_bir_lowering=False,
        debug=False,
        num_devices=NCORES,
    )
    # a16: adjacency shard, transposed to [source, target] fp16, with two
    # source-chunks packed per DRAM row so every DMA descriptor moves a
    # contiguous 8 KiB run per partition (4 KiB runs halve DMA throughput:
    # the SDMA per-packet overhead is ~12 ns regardless of size).
    #   a16[g2*128 + p, half*TSH + t] = fp16(adj[t0 + t, (2*g2 + half)*128 + p])
    a16 = nc.dram_tensor(
        "a16", [N // 2, 2 * TSH], mybir.dt.float16, kind="ExternalInput"
    ).ap()
    # spk: stationary weights, packed [P, SCHUNKS*32] fp16 where
    #   spk[p, n*32 + b]      = fp16_hi(spikes[b, n*128 + p])
    #   spk[p, n*32 + 16 + b] = fp16_lo(spikes[b, n*128 + p])
    spk = nc.dram_tensor(
        "spk", [P, SCHUNKS * 32], mybir.dt.float16, kind="ExternalInput"
    ).ap()
    out = nc.dram_tensor("o", [32, TSH], mybir.dt.float32, kind="ExternalOutput").ap()

    f32 = mybir.dt.float32
    f16 = mybir.dt.float16
    NJ = TSH // NFREE  # 4 PSUM banks

    with tile.TileContext(nc) as tc:
        with (
            tc.tile_pool(name="adj", bufs=5) as adj_pool,
            tc.tile_pool(name="spkp", bufs=1) as spk_pool,
            tc.tile_pool(name="psum", bufs=1, space="PSUM") as psum_pool,
            tc.tile_pool(name="outp", bufs=1) as out_pool,
        ):
            # Stationary weights load first on the same HWDGE queue as the
            # adjacency stream (1 MiB = ~2.4 us serial cost; cheaper than the
            # SWDGE path, whose packets get starved behind the saturated
            # HWDGE queue and delay the first matmul by ~10 us).
            spk_t = spk_pool.tile([P, SCHUNKS * 32], f16)
            nc.sync.dma_start(spk_t[:], spk[:])

            ps = psum_pool.tile([32, TSH], f32)

            ngroups = SCHUNKS // GROUP
            rows = (GROUP // 2) * P  # packed DRAM rows per group
            for g in range(ngroups):
                at = adj_pool.tile([P, GROUP * TSH], f16)
                # Single HWDGE queue: splitting across the SP and ACT rings
                # makes each SDMA engine alternate queues per packet, which
                # costs ~15% engine utilization (measured 320 vs 422 GB/s).
                nc.sync.dma_start(
                    at[:].rearrange("p (n t) -> p n t", n=GROUP // 2),
                    a16[g * rows : (g + 1) * rows, :].rearrange(
                        "(n p) t -> p n t", p=P
                    ),
                )
                for nl in range(GROUP):
                    n = g * GROUP + nl
                    w = spk_t[:, n * 32 : (n + 1) * 32]
                    base = nl * TSH
                    for j in range(NJ):
                        c0 = base + j * NFREE
                        nc.tensor.matmul(
                            ps[:, j * NFREE : (j + 1) * NFREE],
                            w,
                            at[:, c0 : c0 + NFREE],
                            start=(n == 0),
                            stop=(n == SCHUNKS - 1),
                        )

            ot = out_pool.tile([32, TSH], f32)
            nc.vector.tensor_copy(ot[:], ps[:])
            nc.sync.dma_start(out[:], ot[:])

    nc.compile()
    return nc


def _split_hi_lo(x32):
    """Split fp32 array into (hi, lo) fp16 parts with x32 ~= hi + lo."""
    hi = x32.astype(np.float16)
    lo = (x32 - hi.astype(np.float32)).astype(np.float16)
    return hi, lo


def _prep_inputs(spikes, adjacency):
    flat = np.ascontiguousarray(np.asarray(spikes, dtype=np.float32).reshape(B, N))
    adj = np.asarray(adjacency, dtype=np.float32)

    flatT = np.ascontiguousarray(flat.T)  # [N, B]
    fhi, flo = _split_hi_lo(flatT)
    spk = np.empty((SCHUNKS, P, 32), np.float16)  # [n, p, 2*B]
    spk[:, :, :B] = fhi.reshape(SCHUNKS, P, B)
    spk[:, :, B:] = flo.reshape(SCHUNKS, P, B)
    spk = np.ascontiguousarray(spk.transpose(1, 0, 2)).reshape(P, SCHUNKS * 32)

    adjT = adj.T  # [source, target] view (strided)
    in_maps = []
    for i in range(NCORES):
        a16 = adjT[:, i * TSH : (i + 1) * TSH].astype(np.float16)  # [N, TSH]
        # Pack two source-chunks per DRAM row (see kernel comment).
        a16 = np.ascontiguousarray(
            a16.reshape(N // (2 * P), 2, P, TSH).transpose(0, 2, 1, 3)
        ).reshape(N // 2, 2 * TSH)
        in_maps.append({"a16": a16, "spk": spk})
    return in_maps


def _run(in_maps, **kwargs):
    from concourse.bass_utils import run_bass_kernel_spmd

    if "nc" not in _cache:
        _cache["nc"] = _build_nc()
    return run_bass_kernel_spmd(
        _cache["nc"], in_maps, core_ids=list(range(NCORES)), **kwargs
    )


def kernel(spikes, adjacency):
    in_maps = _prep_inputs(spikes, adjacency)
    res = _run(in_maps)
    outs = [r["o"] for r in res.results]
    # Fold hi-weight rows (0:16) + lo-weight rows (16:32), concat target shards.
    full = np.concatenate([o[:B] + o[B:] for o in outs], axis=1)  # [B, N]
    return np.ascontiguousarray(full.reshape(B, H, W), dtype=np.float32)
